# revision 1
# baseline (speedup 1.0000x reference)
"""GAT self-attention Trainium2 kernel.

Full inputs -> shard graphs over 8 NeuronCores -> full output.

Math (per graph n, reference reformulated):
  g_i = sigmoid(relu(q @ W1_i) @ W2_i)            [2d]
  u_i^L = W_i @ (g_i[:d] * a_i[:d])               [k]   (left projector)
  u_i^R = W_i @ (g_i[d:] * a_i[d:])               [k]   (right projector)
  left_i = X @ u_i^L ; right_i = X @ u_i^R        [E]
  score[i,j] = lrelu(left_t[i] + right_t[j]), t = adj[i,j]; -BIG if adj==0
  E = exp(score); rs = rowsum(E); Xs = X / rs[:,None]
  out = (E^T @ Xs) @ W_2          (== softmax(score)^T @ (X @ W_2))
"""
import numpy as np
from contextlib import ExitStack

import concourse.bass as bass
import concourse.tile as tile
from concourse import mybir, bacc
from concourse.masks import make_identity

F32 = mybir.dt.float32
F32R = mybir.dt.float32r
U8 = mybir.dt.uint8
I32 = mybir.dt.int32
AF = mybir.ActivationFunctionType
OP = mybir.AluOpType

N_CORES = 8
N, E, K, D = 64, 512, 512, 512   # graphs, entities, in_dim, out_dim
NG = N // N_CORES                # graphs per core
NT = 3                           # edge types
P = 128
EC = E // P                      # 4 partition chunks of E
KC = K // P
DC2 = (2 * D) // P               # 8 chunks of the 2d gate dim
NEG_BIG = -200.0
LRELU_SLOPE = 0.2
USE_HW_LRELU = True   # ACT Lrelu not implemented in CoreSim; set False for sim runs



def _dma_split(nc, dst, src, pieces):
    """Split a big load along the leading src dim across sync/scalar queues."""
    n0 = dst.shape[1]
    step = max(1, n0 // pieces)
    engs = [nc.sync, nc.scalar]
    i = 0
    c = 0
    while i < n0:
        j = min(n0, i + step)
        engs[c % 2].dma_start(dst[:, i:j], src[:, i:j])
        i = j
        c += 1

def build(nc, reps=1):
    x = nc.dram_tensor("x", [NG, E, K], F32R, kind="ExternalInput").ap()
    adj = nc.dram_tensor("adj", [NG, E, E], I32, kind="ExternalInput").ap()
    qv = nc.dram_tensor("qv", [NG, K], F32R, kind="ExternalInput").ap()
    Wt = nc.dram_tensor("Wt", [NT, K, D], F32R, kind="ExternalInput").ap()
    at = nc.dram_tensor("at", [NT, 2 * D], F32, kind="ExternalInput").ap()
    W1 = nc.dram_tensor("W1", [NT, K, 2 * D], F32R, kind="ExternalInput").ap()
    W2q = nc.dram_tensor("W2q", [NT, 2 * D, 2 * D], F32R, kind="ExternalInput").ap()
    out = nc.dram_tensor("out", [NG, E, D], F32, kind="ExternalOutput").ap()
    nc._gat_io = (x, adj, qv, Wt, at, W1, W2q, out)

    _build_once(nc, reps)


def _build_once(nc, reps=1):
    x, adj, qv, Wt, at, W1, W2q, out = nc._gat_io
    with tile.TileContext(nc) as tc, ExitStack() as ctx:
        # ---------------- persistent pools ----------------
        pers = ctx.enter_context(tc.tile_pool(name="pers", bufs=1))
        ident = pers.tile([P, P], F32)
        make_identity(nc, ident[:])
        ones_stage = pers.tile([1, E], F32)
        nc.vector.memset(ones_stage[:], 1.0)
        ones_row = pers.tile([1, E], F32R)
        nc.vector.tensor_copy(ones_row[:], ones_stage[:])
        neg_col = pers.tile([P, 1], F32)
        nc.vector.memset(neg_col[:], NEG_BIG)
        # U_all[k%128, kc, c, n]: c in 0..2 -> left type c+1, 3..5 -> right
        U_all = pers.tile([P, KC, 2 * NT, NG], F32R)
        Wt2_sb = pers.tile([P, KC, D], F32R)
        _dma_split(nc, Wt2_sb[:], Wt[2].rearrange("(c p) d -> p c d", p=P), 2)

        # ---------------- prep phase ----------------
        def run_prep():
          with tc.tile_pool(name="prep", bufs=1) as prep:
            # qT[k%128, kc, n] via PE transposes of the natural [NG, K] layout
            qv_nat = prep.tile([NG, K], F32R)
            nc.sync.dma_start(qv_nat[:], qv)
            qT = prep.tile([P, KC, NG], F32R)
            for kc in range(KC):
                qps = ps_v.tile([P, NG], F32, tag="v")
                nc.tensor.transpose(
                    qps[:], qv_nat[:, kc * P:(kc + 1) * P].bitcast(F32), ident[:NG, :NG])
                nc.vector.tensor_copy(qT[:, kc, :], qps[:])
            # aT[d2%128, dc2]  (2d = 1024)
            aT = prep.tile([P, DC2, NT], F32)
            with nc.allow_non_contiguous_dma(reason="small aT load"):
                for t in range(NT):
                    nc.sync.dma_start(aT[:, :, t:t + 1],
                                      at[t].rearrange("(c p) -> p c", p=P)[:, :, None])

            for i in range(NT):
                # rrT = relu(W1_i^T @ qT): [2d, NG] laid out [128, DC2, NG]
                rrT = prep.tile([P, DC2, NG], F32R, tag="rrT")
                for whalf in range(2):
                    W1_sb = prep.tile([P, KC, D], F32R, tag="w1")
                    _dma_split(nc, W1_sb[:],
                               W1[i, :, whalf * D:(whalf + 1) * D].rearrange(
                                   "(c p) f -> p c f", p=P), 4)
                    for oc in range(DC2 // 2):
                        oc_g = whalf * (DC2 // 2) + oc
                        pps = ps_v.tile([P, NG], F32, tag="v")
                        for kc in range(KC):
                            nc.tensor.matmul(
                                pps[:], W1_sb[:, kc, oc * P:(oc + 1) * P],
                                qT[:, kc, :],
                                start=(kc == 0), stop=(kc == KC - 1))
                        nc.scalar.activation(rrT[:, oc_g, :], pps[:], AF.Relu)
                # gT = sigmoid(W2q_i^T @ rrT), W2q loaded in two out-halves
                gvT = prep.tile([P, DC2, NG], F32, tag="gvT")
                for half in range(2):
                    W2_sb = prep.tile([P, DC2, D], F32R, tag="w2")
                    _dma_split(
                        nc, W2_sb[:],
                        W2q[i, :, half * D:(half + 1) * D].rearrange(
                            "(c p) f -> p c f", p=P), 4)
                    for oc in range(DC2 // 2):
                        oc_g = half * (DC2 // 2) + oc
                        pps = ps_v.tile([P, NG], F32, tag="v")
                        for dc in range(DC2):
                            nc.tensor.matmul(
                                pps[:], W2_sb[:, dc, oc * P:(oc + 1) * P],
                                rrT[:, dc, :],
                                start=(dc == 0), stop=(dc == DC2 - 1))
                        nc.scalar.activation(gvT[:, oc_g, :], pps[:], AF.Sigmoid)
                # vT = gT * aT_i  (per-element over the 2d axis, bcast over n)
                vT = prep.tile([P, DC2, NG], F32R, tag="vT")
                nc.vector.tensor_tensor(
                    vT[:], gvT[:], aT[:, :, i:i + 1].broadcast_to((P, DC2, NG)),
                    OP.mult)
                # WT_i = W_i^T via PE transposes: [d%128, dc, k]
                W_sb = prep.tile([P, KC, D], F32R, tag="wsb")
                _dma_split(nc, W_sb[:], Wt[i].rearrange("(c p) d -> p c d", p=P), 2)
                WTi = prep.tile([P, EC, K], F32R, tag="wti")
                for dc in range(EC):
                    tps = ps_tr.tile([P, E], F32, tag="tr")
                    for kc in range(KC):
                        nc.tensor.transpose(
                            tps[:, kc * P:(kc + 1) * P],
                            W_sb[:, kc, dc * P:(dc + 1) * P].bitcast(F32), ident[:])
                    nc.vector.tensor_copy(WTi[:, dc, :], tps[:])
                # U_i(side) = W_i @ v-half : contraction over d
                for s in range(2):
                    ups = ps_v.tile([P, KC, NG], F32, tag="v")
                    for kc in range(KC):
                        for dc in range(EC):
                            nc.tensor.matmul(
                                ups[:, kc, :],
                                WTi[:, dc, kc * P:(kc + 1) * P],
                                vT[:, s * EC + dc, :],
                                start=(dc == 0), stop=(dc == EC - 1))
                    # c index: left types at 0..2, right at 3..5 (c = 3*s + i)
                    nc.vector.tensor_copy(U_all[:, :, 3 * s + i, :], ups[:])

        # ---------------- main per-graph pipeline ----------------
        sbuf = ctx.enter_context(tc.tile_pool(name="sbuf", bufs=2))
        deep = ctx.enter_context(tc.tile_pool(name="deep", bufs=3))
        small = ctx.enter_context(tc.tile_pool(name="small", bufs=2))
        one = ctx.enter_context(tc.tile_pool(name="one", bufs=1))
        ps_big = ctx.enter_context(tc.tile_pool(name="ps_big", bufs=2, space="PSUM"))
        ps_v = ctx.enter_context(tc.tile_pool(name="ps_v", bufs=4, space="PSUM"))
        ps_tr = ctx.enter_context(tc.tile_pool(name="ps_tr", bufs=1, space="PSUM"))
        ps_lr = ctx.enter_context(tc.tile_pool(name="ps_lr", bufs=1, space="PSUM"))

        def phase1(n):
            """front half: inputs, Xt, LR rows, stacks, masks"""
            X_sb = deep.tile([P, EC, K], F32R, tag="X")
            nc.sync.dma_start(X_sb[:, 0:2], x[n].rearrange("(c p) k -> p c k", p=P)[:, 0:2])
            nc.scalar.dma_start(X_sb[:, 2:4], x[n].rearrange("(c p) k -> p c k", p=P)[:, 2:4])
            adj_sb = sbuf.tile([P, EC, E], I32, tag="adj")
            nc.scalar.dma_start(adj_sb[:, 0:2], adj[n].rearrange("(c p) j -> p c j", p=P)[:, 0:2])
            nc.sync.dma_start(adj_sb[:, 2:4], adj[n].rearrange("(c p) j -> p c j", p=P)[:, 2:4])

            Xt_sb = sbuf.tile([P, KC, E], F32R, tag="Xt")
            for kc in range(KC):
                tps = ps_tr.tile([P, E], F32, tag="tr")
                for ec in range(EC):
                    nc.tensor.transpose(
                        tps[:, ec * P:(ec + 1) * P],
                        X_sb[:, ec, kc * P:(kc + 1) * P].bitcast(F32), ident[:])
                nc.scalar.copy(Xt_sb[:, kc, :], tps[:])

            pLR = ps_lr.tile([2 * NT, E], F32, tag="lr")
            for kc in range(KC):
                nc.tensor.matmul(pLR[:], U_all[:, kc, :, n], Xt_sb[:, kc, :],
                                 start=(kc == 0), stop=(kc == KC - 1))
            LR_sb = small.tile([2 * NT, E], F32R, tag="lrs")
            nc.scalar.copy(LR_sb[:], pLR[:])

            lhsT = []
            rhsT = []
            for t in range(NT):
                eng_a = nc.sync if t % 2 == 0 else nc.scalar
                eng_b = nc.scalar if t % 2 == 0 else nc.sync
                lt = small.tile([2, E], F32R, tag=f"lt{t}")
                eng_a.dma_start(lt[0:1, :], ones_row[:])
                eng_b.dma_start(lt[1:2, :], LR_sb[t:t + 1, :])
                rt = small.tile([2, E], F32R, tag=f"rt{t}")
                eng_a.dma_start(rt[0:1, :], LR_sb[NT + t:NT + t + 1, :])
                eng_b.dma_start(rt[1:2, :], ones_row[:])
                lhsT.append(lt)
                rhsT.append(rt)

            m0 = sbuf.tile([P, EC, E], U8, tag="m0")
            m2 = sbuf.tile([P, EC, E], U8, tag="m2")
            m3 = sbuf.tile([P, EC, E], U8, tag="m3")
            for h in range(2):
                sl = slice(2 * h, 2 * h + 2)
                nc.gpsimd.tensor_scalar(m2[:, sl], adj_sb[:, sl], 2, None, OP.is_equal)
                nc.gpsimd.tensor_scalar(m3[:, sl], adj_sb[:, sl], 3, None, OP.is_equal)
                nc.gpsimd.tensor_scalar(m0[:, sl], adj_sb[:, sl], 0, None, OP.is_equal)
            return dict(X_sb=X_sb, lhsT=lhsT, rhsT=rhsT, m0=m0, m2=m2, m3=m3)

        def phase2(n, st):
            """back half: select, exp, F, out"""
            X_sb = st["X_sb"]; lhsT = st["lhsT"]; rhsT = st["rhsT"]
            m0 = st["m0"]; m2 = st["m2"]; m3 = st["m3"]
            E_sb = deep.tile([P, EC, E], F32R, tag="E")
            rs = small.tile([P, EC], F32, tag="rs")
            for ic in range(EC):
                pv = []
                for t in range(NT):
                    pvt = ps_v.tile([P, E], F32, tag="v")
                    nc.tensor.matmul(pvt[:], lhsT[t][:, ic * P:(ic + 1) * P],
                                     rhsT[t][:], start=True, stop=True)
                    pv.append(pvt)
                nc.vector.copy_predicated(pv[0][:], m2[:, ic, :], pv[1][:])
                nc.vector.copy_predicated(pv[0][:], m3[:, ic, :], pv[2][:])
                nc.vector.copy_predicated(pv[0][:], m0[:, ic, :],
                                          neg_col[:, 0:1].broadcast_to((P, E)))
                ab = small.tile([P, E], F32, tag="ab")
                nc.scalar.activation(ab[:], pv[0][:], AF.Abs, scale=0.4)
                sc = small.tile([P, E], F32, tag="sc")
                nc.vector.scalar_tensor_tensor(sc[:], pv[0][:], 0.6, ab[:],
                                               OP.mult, OP.add)
                nc.scalar.activation(E_sb[:, ic, :], sc[:], AF.Exp,
                                     accum_out=rs[:, ic:ic + 1])
                rsr_ic = small.tile([P, EC], F32, tag="rsr")
                nc.vector.reciprocal(rsr_ic[:, ic:ic + 1], rs[:, ic:ic + 1])
                nc.vector.tensor_scalar(E_sb[:, ic, :], E_sb[:, ic, :].bitcast(F32),
                                        rsr_ic[:, ic:ic + 1], None, OP.mult)

            F_sb = sbuf.tile([P, KC, E], F32R, tag="F")
            for kc in range(KC):
                pF = ps_big.tile([P, E], F32, tag="big")
                for ec in range(EC):
                    nc.tensor.matmul(pF[:], X_sb[:, ec, kc * P:(kc + 1) * P],
                                     E_sb[:, ec, :],
                                     start=(ec == 0), stop=(ec == EC - 1))
                nc.scalar.copy(F_sb[:, kc, :], pF[:])

            for jc in range(EC):
                pO = ps_big.tile([P, D], F32, tag="big")
                for kc in range(KC):
                    nc.tensor.matmul(pO[:], F_sb[:, kc, jc * P:(jc + 1) * P],
                                     Wt2_sb[:, kc, :],
                                     start=(kc == 0), stop=(kc == KC - 1))
                o_sb = small.tile([P, D], F32, tag="osb")
                nc.scalar.copy(o_sb[:], pO[:])
                (nc.sync if jc % 2 == 0 else nc.scalar).dma_start(
                    out[n, jc * P:(jc + 1) * P, :], o_sb[:])

        def body_all(_iv=None):
          run_prep()
          for n in range(NG):
              phase2(n, phase1(n))

        if reps == 1:
            body_all()
        else:
            with tc.For_i(0, reps, 1) as _iv:
                body_all(_iv)
    return nc


_NC_CACHE = {}
TRACE = False
_LAST = {}


def _get_nc():
    if "nc" not in _NC_CACHE:
        nc = bacc.Bacc("TRN2", target_bir_lowering=False, debug=False)
        build(nc)
        nc.compile()
        _NC_CACHE["nc"] = nc
    return _NC_CACHE["nc"]


def kernel(input_state, adj, entity_mask, query_vec, W_type, a_type,
           qattn_W1, qattn_W2):
    from concourse import bass_utils
    nc = _get_nc()
    input_state = np.ascontiguousarray(input_state, dtype=np.float32)
    adj = np.ascontiguousarray(adj, dtype=np.int32)
    query_vec = np.ascontiguousarray(query_vec, dtype=np.float32)
    W_type = np.ascontiguousarray(W_type, dtype=np.float32)
    a_type = np.ascontiguousarray(a_type, dtype=np.float32)
    qattn_W1 = np.ascontiguousarray(qattn_W1, dtype=np.float32)
    qattn_W2 = np.ascontiguousarray(qattn_W2, dtype=np.float32)

    in_maps = []
    for c in range(N_CORES):
        sl = slice(c * NG, (c + 1) * NG)
        in_maps.append({
            "x": input_state[sl], "adj": adj[sl], "qv": query_vec[sl],
            "Wt": W_type, "at": a_type, "W1": qattn_W1, "W2q": qattn_W2,
        })
    res = bass_utils.run_bass_kernel_spmd(nc, in_maps, core_ids=list(range(N_CORES)),
                                          trace=TRACE, stitch_traces=TRACE)
    _LAST["exec_ns"] = res.exec_time_ns
    _LAST["mean_ns"] = res.mean_exec_time_ns
    _LAST["trace"] = res.instructions_and_trace
    _LAST["scope_times"] = res.per_core_scope_times
    out = np.concatenate([r["out"] for r in res.results], axis=0)
    return out.astype(np.float32)



# revision 4
# speedup vs baseline: 1.1236x; 1.1236x over previous
"""GAT self-attention Trainium2 kernel (v2: bf16 datapath, overlapped prep).

Full inputs -> shard graphs over 8 NeuronCores -> full output.

Math (per graph n, reference reformulated):
  g_i = sigmoid(relu(q @ W1_i) @ W2_i)            [2d]
  u_i^L = W_i @ (g_i[:d] * a_i[:d])               [k]   (left projector)
  u_i^R = W_i @ (g_i[d:] * a_i[d:])               [k]   (right projector)
  left_i = X @ u_i^L ; right_i = X @ u_i^R        [E]
  score[i,j] = lrelu(left_t[i] + right_t[j]), t = adj[i,j]; -BIG if adj==0
  E = exp(score); rs = rowsum(E); En = E / rs[:,None]
  out = (En^T @ X) @ W_2

Host marshaling: inputs/weights cast to bf16 (adj to int8, lossless); weight
matrices pre-transposed so the device never transposes weights; query vectors
pre-packed in transposed layout. Device does all matmuls/softmax; output is
written fp32 straight from PSUM.
"""
import numpy as np
from contextlib import ExitStack

import concourse.bass as bass
import concourse.tile as tile
from concourse import mybir, bacc
from concourse.masks import make_identity

F32 = mybir.dt.float32
F32R = mybir.dt.float32r
BF16 = mybir.dt.bfloat16
U8 = mybir.dt.uint8
I8 = mybir.dt.int8
AF = mybir.ActivationFunctionType
OP = mybir.AluOpType

N_CORES = 8
N, E, K, D = 64, 512, 512, 512   # graphs, entities, in_dim, out_dim
NG = N // N_CORES                # graphs per core
NT = 3                           # edge types
P = 128
EC = E // P                      # 4 partition chunks of E
KC = K // P
DC2 = (2 * D) // P               # 8 chunks of the 2d gate dim
NEG_BIG = -200.0
LRELU_SLOPE = 0.2
USE_ACT_LRELU = True             # leaky-relu on ACT engine (alpha operand)


def _dma_split(nc, engs, dst, src, pieces):
    """Split a load along dim 1 of dst across the given engine queues."""
    n0 = dst.shape[1]
    step = max(1, n0 // pieces)
    i = 0
    c = 0
    while i < n0:
        j = min(n0, i + step)
        engs[c % len(engs)].dma_start(dst[:, i:j], src[:, i:j])
        i = j
        c += 1


def build(nc, reps=1):
    x = nc.dram_tensor("x", [NG, E, K], BF16, kind="ExternalInput").ap()
    adj = nc.dram_tensor("adj", [NG, E, E], I8, kind="ExternalInput").ap()
    qT = nc.dram_tensor("qT", [P, KC, NG], BF16, kind="ExternalInput").ap()
    aT = nc.dram_tensor("aT", [P, DC2, NT], F32, kind="ExternalInput").ap()
    WtT = nc.dram_tensor("WtT", [NT, D, K], BF16, kind="ExternalInput").ap()
    Wt2 = nc.dram_tensor("Wt2", [K, D], BF16, kind="ExternalInput").ap()
    W1 = nc.dram_tensor("W1", [NT, K, 2 * D], BF16, kind="ExternalInput").ap()
    W2q = nc.dram_tensor("W2q", [NT, 2 * D, 2 * D], BF16, kind="ExternalInput").ap()
    out = nc.dram_tensor("out", [NG, E, D], BF16, kind="ExternalOutput").ap()
    nc._gat_io = (x, adj, qT, aT, WtT, Wt2, W1, W2q, out)
    _build_once(nc, reps)


def _build_once(nc, reps=1):
    x, adj, qT_d, aT_d, WtT, Wt2, W1, W2q, out = nc._gat_io
    with tile.TileContext(nc) as tc, ExitStack() as ctx:
        # ---------------- pools ----------------
        pers = ctx.enter_context(tc.tile_pool(name="pers", bufs=1))
        prep = ctx.enter_context(tc.tile_pool(name="prep", bufs=2))
        deep = ctx.enter_context(tc.tile_pool(name="deep", bufs=3))
        sbuf = ctx.enter_context(tc.tile_pool(name="sbuf", bufs=2))
        small = ctx.enter_context(tc.tile_pool(name="small", bufs=2))
        ps_tr = ctx.enter_context(tc.tile_pool(name="ps_tr", bufs=1, space="PSUM"))
        ps_lr = ctx.enter_context(tc.tile_pool(name="ps_lr", bufs=1, space="PSUM"))
        ps_v = ctx.enter_context(tc.tile_pool(name="ps_v", bufs=4, space="PSUM"))
        ps_big = ctx.enter_context(tc.tile_pool(name="ps_big", bufs=2, space="PSUM"))

        # ---------------- persistent tiles ----------------
        identB = pers.tile([P, P], BF16)
        make_identity(nc, identB[:])
        neg_col = pers.tile([P, 1], F32)
        nc.vector.memset(neg_col[:], NEG_BIG)
        # U_all[k%128, kc, s, i, n]: projectors, order (L1,L2,L3,R1,R2,R3)
        U_all = pers.tile([P, KC, 2, NT, NG], F32R)
        qT_sb = pers.tile([P, KC, NG], BF16)
        aT_sb = pers.tile([P, DC2, NT], F32)
        Wt2_sb = pers.tile([P, KC, D], BF16)

        def phase1(n):
            """weight-independent front half: loads, Xt, masks"""
            X_sb = deep.tile([P, EC, K], BF16, tag="X")
            nc.sync.dma_start(X_sb[:, 0:2], x[n].rearrange("(c p) k -> p c k", p=P)[:, 0:2])
            nc.sync.dma_start(X_sb[:, 2:4], x[n].rearrange("(c p) k -> p c k", p=P)[:, 2:4])
            adj_sb = deep.tile([P, EC, E], I8, tag="adj")
            nc.sync.dma_start(adj_sb[:], adj[n].rearrange("(c p) j -> p c j", p=P))

            Xt_sb = deep.tile([P, KC, E], BF16, tag="Xt")
            for kc in range(KC):
                tps = ps_tr.tile([P, E], BF16, tag="tr")
                for ec in range(EC):
                    nc.tensor.transpose(
                        tps[:, ec * P:(ec + 1) * P],
                        X_sb[:, ec, kc * P:(kc + 1) * P], identB[:])
                nc.vector.tensor_copy(Xt_sb[:, kc, :], tps[:])

            m0 = deep.tile([P, EC, E], U8, tag="m0")
            m2 = deep.tile([P, EC, E], U8, tag="m2")
            m3 = deep.tile([P, EC, E], U8, tag="m3")
            nc.gpsimd.tensor_scalar(m0[:], adj_sb[:], 0, None, OP.is_equal)
            nc.gpsimd.tensor_scalar(m2[:], adj_sb[:], 2, None, OP.is_equal)
            nc.gpsimd.tensor_scalar(m3[:], adj_sb[:], 3, None, OP.is_equal)
            return dict(X_sb=X_sb, Xt_sb=Xt_sb, m0=m0, m2=m2, m3=m3)

        def run_prep():
            nc.scalar.dma_start(qT_sb[:], qT_d)
            nc.scalar.dma_start(aT_sb[:], aT_d)
            for i in range(NT):
                # weight loads first so DMA stays saturated
                W1_sb = prep.tile([P, KC, 2 * D], BF16, tag="w1")
                _dma_split(nc, [nc.scalar], W1_sb[:],
                           W1[i].rearrange("(c p) f -> p c f", p=P), 2)
                W2_sb = prep.tile([P, DC2, 2 * D], BF16, tag="w2")
                _dma_split(nc, [nc.scalar], W2_sb[:],
                           W2q[i].rearrange("(c p) f -> p c f", p=P), 4)
                WT_sb = prep.tile([P, EC, K], BF16, tag="wt")
                _dma_split(nc, [nc.scalar], WT_sb[:],
                           WtT[i].rearrange("(c p) k -> p c k", p=P), 2)

                # rr = relu(q @ W1_i):  [NG, 2d] in two 512-halves
                rr_sb = prep.tile([NG, 2 * D], BF16, tag="rr")
                for half in range(2):
                    rp = ps_v.tile([NG, D], F32, tag="v")
                    for kc in range(KC):
                        nc.tensor.matmul(
                            rp[:], qT_sb[:, kc, :],
                            W1_sb[:, kc, half * D:(half + 1) * D],
                            start=(kc == 0), stop=(kc == KC - 1))
                    nc.scalar.activation(rr_sb[:, half * D:(half + 1) * D],
                                         rp[:], AF.Relu)
                # rrT[(2d)%128, dc, n] via PE transposes
                rrT = prep.tile([P, DC2, NG], BF16, tag="rrT")
                trp = ps_tr.tile([P, DC2, NG], BF16, tag="tr")
                for dc in range(DC2):
                    nc.tensor.transpose(trp[:, dc, :],
                                        rr_sb[:, dc * P:(dc + 1) * P],
                                        identB[:NG, :NG])
                nc.vector.tensor_copy(rrT[:], trp[:])
                # gv = sigmoid(rr @ W2_i)
                gv_sb = prep.tile([NG, 2 * D], BF16, tag="gv")
                for half in range(2):
                    gp = ps_v.tile([NG, D], F32, tag="v")
                    for dc in range(DC2):
                        nc.tensor.matmul(
                            gp[:], rrT[:, dc, :],
                            W2_sb[:, dc, half * D:(half + 1) * D],
                            start=(dc == 0), stop=(dc == DC2 - 1))
                    nc.scalar.activation(gv_sb[:, half * D:(half + 1) * D],
                                         gp[:], AF.Sigmoid)
                # gvT then v = gv * a_i  (broadcast over n)
                trp2 = ps_tr.tile([P, DC2, NG], BF16, tag="tr")
                for dc in range(DC2):
                    nc.tensor.transpose(trp2[:, dc, :],
                                        gv_sb[:, dc * P:(dc + 1) * P],
                                        identB[:NG, :NG])
                vT = prep.tile([P, DC2, NG], BF16, tag="vT")
                nc.vector.tensor_tensor(
                    vT[:], trp2[:],
                    aT_sb[:, :, i:i + 1].broadcast_to((P, DC2, NG)), OP.mult)
                # U_i(side) = W_i^T-contracted projectors, both sides at once:
                # lhsT = WtT_i chunk [d,128k], rhs = vT[:, {dc, dc+4}, :]
                for kc in range(KC):
                    up = ps_v.tile([P, 2, NG], F32, tag="v")
                    for dc in range(EC):
                        nc.tensor.matmul(
                            up[:], WT_sb[:, dc, kc * P:(kc + 1) * P],
                            vT[:, dc:dc + EC + 1:EC, :],
                            start=(dc == 0), stop=(dc == EC - 1))
                    nc.vector.tensor_copy(U_all[:, kc, :, i, :], up[:])
            _dma_split(nc, [nc.scalar], Wt2_sb[:],
                       Wt2.rearrange("(c p) d -> p c d", p=P), 2)
            # prefill the ones rows of both ring buffers of the LR stacks
            for _ in range(2):
                Lt = small.tile([66, E], F32R, tag="Lt")
                nc.vector.memset(Lt[0:65:32, :].bitcast(F32), 1.0)
                Rt = small.tile([66, E], F32R, tag="Rt")
                nc.vector.memset(Rt[1:66:32, :].bitcast(F32), 1.0)

        def phase2(n, st):
            """back half: LR rows, scores, softmax, F, out"""
            X_sb = st["X_sb"]; Xt_sb = st["Xt_sb"]
            m0 = st["m0"]; m2 = st["m2"]; m3 = st["m3"]

            pLR = ps_lr.tile([2 * NT, E], F32, tag="lr")
            for kc in range(KC):
                nc.tensor.matmul(pLR[:], U_all[:, kc, :, :, n], Xt_sb[:, kc, :],
                                 start=(kc == 0), stop=(kc == KC - 1))
            # stacks: Lt rows {32t: one, 32t+1: L_t}, Rt rows {32t: R_t, 32t+1: one}
            Lt = small.tile([66, E], F32R, tag="Lt")
            Rt = small.tile([66, E], F32R, tag="Rt")
            nc.scalar.copy(Lt[1:66:32, :], pLR[0:NT, :])
            nc.scalar.copy(Rt[0:65:32, :], pLR[NT:2 * NT, :])

            E_sb = sbuf.tile([P, EC, E], BF16, tag="E")
            rs = small.tile([P, EC], F32, tag="rs")
            rsr = small.tile([P, EC], F32, tag="rsr")
            for ic in range(EC):
                pv = []
                for t in range(NT):
                    pvt = ps_v.tile([P, E], F32, tag="v")
                    nc.tensor.matmul(pvt[:], Lt[32 * t:32 * t + 2, ic * P:(ic + 1) * P],
                                     Rt[32 * t:32 * t + 2, :], start=True, stop=True)
                    pv.append(pvt)
                nc.vector.copy_predicated(pv[0][:], m2[:, ic, :], pv[1][:])
                nc.vector.copy_predicated(pv[0][:], m3[:, ic, :], pv[2][:])
                nc.vector.copy_predicated(pv[0][:], m0[:, ic, :],
                                          neg_col[:, 0:1].broadcast_to((P, E)))
                sc = small.tile([P, E], F32, tag="sc")
                if USE_ACT_LRELU:
                    nc.scalar.activation(sc[:], pv[0][:], AF.Lrelu,
                                         alpha=LRELU_SLOPE)
                else:
                    nc.vector.scalar_tensor_tensor(sc[:], pv[0][:], LRELU_SLOPE,
                                                   pv[0][:], OP.mult, OP.max)
                nc.scalar.activation(E_sb[:, ic, :], sc[:], AF.Exp,
                                     accum_out=rs[:, ic:ic + 1])
                nc.vector.reciprocal(rsr[:, ic:ic + 1], rs[:, ic:ic + 1])
                nc.vector.tensor_scalar(E_sb[:, ic, :], E_sb[:, ic, :],
                                        rsr[:, ic:ic + 1], None, OP.mult)

            F_sb = sbuf.tile([P, KC, E], BF16, tag="F")
            for kc in range(KC):
                pF = ps_big.tile([P, E], F32, tag="big")
                for ec in range(EC):
                    nc.tensor.matmul(pF[:], X_sb[:, ec, kc * P:(kc + 1) * P],
                                     E_sb[:, ec, :],
                                     start=(ec == 0), stop=(ec == EC - 1))
                nc.scalar.copy(F_sb[:, kc, :], pF[:])

            for jc in range(EC):
                pO = ps_big.tile([P, D], F32, tag="big")
                for kc in range(KC):
                    nc.tensor.matmul(pO[:], F_sb[:, kc, jc * P:(jc + 1) * P],
                                     Wt2_sb[:, kc, :],
                                     start=(kc == 0), stop=(kc == KC - 1))
                o_sb = small.tile([P, D], BF16, tag="osb")
                if jc % 2 == 0:
                    nc.gpsimd.tensor_copy(o_sb[:], pO[:])
                else:
                    nc.scalar.copy(o_sb[:], pO[:])
                nc.sync.dma_start(out[n, jc * P:(jc + 1) * P, :], o_sb[:])

        def body_all(_iv=None):
            sts = {}
            sts[0] = phase1(0)
            sts[1] = phase1(1)
            run_prep()
            for n in range(NG):
                if n + 2 < NG:
                    sts[n + 2] = phase1(n + 2)
                phase2(n, sts.pop(n))

        if reps == 1:
            body_all()
        else:
            with tc.For_i(0, reps, 1) as _iv:
                body_all(_iv)
    return nc


_NC_CACHE = {}
TRACE = False
_LAST = {}


def _get_nc():
    if "nc" not in _NC_CACHE:
        nc = bacc.Bacc("TRN2", target_bir_lowering=False, debug=False)
        build(nc)
        nc.compile()
        _NC_CACHE["nc"] = nc
    return _NC_CACHE["nc"]


def kernel(input_state, adj, entity_mask, query_vec, W_type, a_type,
           qattn_W1, qattn_W2):
    import ml_dtypes
    from concourse import bass_utils
    bf16 = ml_dtypes.bfloat16
    nc = _get_nc()

    x_bf = np.ascontiguousarray(input_state, dtype=np.float32).astype(bf16)
    adj_i8 = np.ascontiguousarray(adj).astype(np.int8)
    qv = np.ascontiguousarray(query_vec, dtype=np.float32).astype(bf16)
    # aT[p, dc2, t] = a_type[t, dc2*128 + p]
    aT = np.ascontiguousarray(
        np.transpose(np.asarray(a_type, np.float32).reshape(NT, DC2, P),
                     (2, 1, 0)))
    WtT = np.ascontiguousarray(
        np.transpose(np.asarray(W_type, np.float32), (0, 2, 1))).astype(bf16)
    Wt2 = np.ascontiguousarray(np.asarray(W_type, np.float32)[2]).astype(bf16)
    W1_bf = np.ascontiguousarray(qattn_W1, dtype=np.float32).astype(bf16)
    W2_bf = np.ascontiguousarray(qattn_W2, dtype=np.float32).astype(bf16)

    in_maps = []
    for c in range(N_CORES):
        sl = slice(c * NG, (c + 1) * NG)
        # qT[p, kc, n] = qv[n, kc*128 + p]
        qT = np.ascontiguousarray(
            np.transpose(qv[sl].reshape(NG, KC, P), (2, 1, 0)))
        in_maps.append({
            "x": x_bf[sl], "adj": adj_i8[sl], "qT": qT, "aT": aT,
            "WtT": WtT, "Wt2": Wt2, "W1": W1_bf, "W2q": W2_bf,
        })
    res = bass_utils.run_bass_kernel_spmd(nc, in_maps, core_ids=list(range(N_CORES)),
                                          trace=TRACE, stitch_traces=TRACE)
    _LAST["exec_ns"] = res.exec_time_ns
    _LAST["mean_ns"] = res.mean_exec_time_ns
    _LAST["trace"] = res.instructions_and_trace
    _LAST["scope_times"] = res.per_core_scope_times
    out = np.concatenate([np.asarray(r["out"], np.float32) for r in res.results],
                         axis=0)
    return out


# revision 7
# speedup vs baseline: 1.4194x; 1.2633x over previous
"""GAT self-attention Trainium2 kernel (v2: bf16 datapath, overlapped prep).

Full inputs -> shard graphs over 8 NeuronCores -> full output.

Math (per graph n, reference reformulated):
  g_i = sigmoid(relu(q @ W1_i) @ W2_i)            [2d]
  u_i^L = W_i @ (g_i[:d] * a_i[:d])               [k]   (left projector)
  u_i^R = W_i @ (g_i[d:] * a_i[d:])               [k]   (right projector)
  left_i = X @ u_i^L ; right_i = X @ u_i^R        [E]
  score[i,j] = lrelu(left_t[i] + right_t[j]), t = adj[i,j]; -BIG if adj==0
  E = exp(score); rs = rowsum(E); En = E / rs[:,None]
  out = (En^T @ X) @ W_2

Host marshaling: inputs/weights cast to bf16 (adj to int8, lossless); weight
matrices pre-transposed so the device never transposes weights; query vectors
pre-packed in transposed layout. Device does all matmuls/softmax; output is
written fp32 straight from PSUM.
"""
import numpy as np
from contextlib import ExitStack

import concourse.bass as bass
import concourse.tile as tile
from concourse import mybir, bacc
from concourse.masks import make_identity

F32 = mybir.dt.float32
F32R = mybir.dt.float32r
BF16 = mybir.dt.bfloat16
U8 = mybir.dt.uint8
I8 = mybir.dt.int8
AF = mybir.ActivationFunctionType
OP = mybir.AluOpType

N_CORES = 8
N, E, K, D = 64, 512, 512, 512   # graphs, entities, in_dim, out_dim
NG = N // N_CORES                # graphs per core
NT = 3                           # edge types
P = 128
EC = E // P                      # 4 partition chunks of E
KC = K // P
DC2 = (2 * D) // P               # 8 chunks of the 2d gate dim
NEG_BIG = -200.0
LRELU_SLOPE = 0.2
USE_ACT_LRELU = True             # leaky-relu on ACT engine (alpha operand)


def _dma_split(nc, engs, dst, src, pieces):
    """Split a load along dim 1 of dst across the given engine queues."""
    n0 = dst.shape[1]
    step = max(1, n0 // pieces)
    i = 0
    c = 0
    while i < n0:
        j = min(n0, i + step)
        engs[c % len(engs)].dma_start(dst[:, i:j], src[:, i:j])
        i = j
        c += 1


def build(nc, reps=1):
    x = nc.dram_tensor("x", [NG, E, K], BF16, kind="ExternalInput").ap()
    adj = nc.dram_tensor("adj", [NG, E, E], I8, kind="ExternalInput").ap()
    qT = nc.dram_tensor("qT", [P, KC, NG], BF16, kind="ExternalInput").ap()
    aT = nc.dram_tensor("aT", [P, DC2, NT], F32, kind="ExternalInput").ap()
    WtT = nc.dram_tensor("WtT", [NT, D, K], BF16, kind="ExternalInput").ap()
    Wt2 = nc.dram_tensor("Wt2", [K, D], BF16, kind="ExternalInput").ap()
    W1 = nc.dram_tensor("W1", [NT, K, 2 * D], BF16, kind="ExternalInput").ap()
    W2q = nc.dram_tensor("W2q", [NT, 2 * D, 2 * D], BF16, kind="ExternalInput").ap()
    out = nc.dram_tensor("out", [NG, E, D], BF16, kind="ExternalOutput").ap()
    nc._gat_io = (x, adj, qT, aT, WtT, Wt2, W1, W2q, out)
    _build_once(nc, reps)


def _build_once(nc, reps=1):
    x, adj, qT_d, aT_d, WtT, Wt2, W1, W2q, out = nc._gat_io
    with tile.TileContext(nc) as tc, ExitStack() as ctx:
        # ---------------- pools ----------------
        pers = ctx.enter_context(tc.tile_pool(name="pers", bufs=1))
        prep = ctx.enter_context(tc.tile_pool(name="prep", bufs=2))
        deep = ctx.enter_context(tc.tile_pool(name="deep", bufs=3))
        sbuf = ctx.enter_context(tc.tile_pool(name="sbuf", bufs=2))
        small = ctx.enter_context(tc.tile_pool(name="small", bufs=2))
        ps_tr = ctx.enter_context(tc.tile_pool(name="ps_tr", bufs=1, space="PSUM"))
        ps_lr = ctx.enter_context(tc.tile_pool(name="ps_lr", bufs=1, space="PSUM"))
        ps_v = ctx.enter_context(tc.tile_pool(name="ps_v", bufs=4, space="PSUM"))
        ps_big = ctx.enter_context(tc.tile_pool(name="ps_big", bufs=2, space="PSUM"))

        # ---------------- persistent tiles ----------------
        identB = pers.tile([P, P], BF16)
        make_identity(nc, identB[:])
        neg_col = pers.tile([P, 1], F32)
        nc.vector.memset(neg_col[:], NEG_BIG)
        # U_all[k%128, kc, s, i, n]: projectors, order (L1,L2,L3,R1,R2,R3)
        U_all = pers.tile([P, KC, 2, NT, NG], F32R)
        qT_sb = pers.tile([P, KC, NG], BF16)
        aT_sb = pers.tile([P, DC2, NT], F32)
        Wt2_sb = pers.tile([P, KC, D], BF16)

        def phase1(n):
            """weight-independent front half: loads, Xt, masks"""
            X_sb = deep.tile([P, EC, K], BF16, tag="X")
            nc.sync.dma_start(X_sb[:, 0:2], x[n].rearrange("(c p) k -> p c k", p=P)[:, 0:2])
            nc.sync.dma_start(X_sb[:, 2:4], x[n].rearrange("(c p) k -> p c k", p=P)[:, 2:4])
            adj_sb = deep.tile([P, EC, E], I8, tag="adj")
            nc.sync.dma_start(adj_sb[:], adj[n].rearrange("(c p) j -> p c j", p=P))

            Xt_sb = deep.tile([P, KC, E], BF16, tag="Xt")
            for kc in range(KC):
                tps = ps_tr.tile([P, E], BF16, tag="tr")
                for ec in range(EC):
                    nc.tensor.transpose(
                        tps[:, ec * P:(ec + 1) * P],
                        X_sb[:, ec, kc * P:(kc + 1) * P], identB[:])
                nc.vector.tensor_copy(Xt_sb[:, kc, :], tps[:])

            m2 = deep.tile([P, EC, E], U8, tag="m2")
            m3 = deep.tile([P, EC, E], U8, tag="m3")
            nc.gpsimd.tensor_scalar(m2[:], adj_sb[:], 2, None, OP.is_equal)
            nc.gpsimd.tensor_scalar(m3[:], adj_sb[:], 3, None, OP.is_equal)
            return dict(X_sb=X_sb, Xt_sb=Xt_sb, adj_sb=adj_sb, m2=m2, m3=m3)

        def run_prep():
            nc.scalar.dma_start(qT_sb[:], qT_d)
            nc.scalar.dma_start(aT_sb[:], aT_d)
            for i in range(NT):
                # weight loads first so DMA stays saturated
                W1_sb = prep.tile([P, KC, 2 * D], BF16, tag="w1")
                _dma_split(nc, [nc.scalar], W1_sb[:],
                           W1[i].rearrange("(c p) f -> p c f", p=P), 2)
                W2_sb = prep.tile([P, DC2, 2 * D], BF16, tag="w2")
                _dma_split(nc, [nc.scalar], W2_sb[:],
                           W2q[i].rearrange("(c p) f -> p c f", p=P), 4)
                WT_sb = prep.tile([P, EC, K], BF16, tag="wt")
                _dma_split(nc, [nc.scalar], WT_sb[:],
                           WtT[i].rearrange("(c p) k -> p c k", p=P), 2)

                # rr = relu(q @ W1_i):  [NG, 2d] in two 512-halves
                rr_sb = prep.tile([NG, 2 * D], BF16, tag="rr")
                for half in range(2):
                    rp = ps_v.tile([NG, D], F32, tag="v")
                    for kc in range(KC):
                        nc.tensor.matmul(
                            rp[:], qT_sb[:, kc, :],
                            W1_sb[:, kc, half * D:(half + 1) * D],
                            start=(kc == 0), stop=(kc == KC - 1))
                    nc.scalar.activation(rr_sb[:, half * D:(half + 1) * D],
                                         rp[:], AF.Relu)
                # rrT[(2d)%128, dc, n] via PE transposes
                rrT = prep.tile([P, DC2, NG], BF16, tag="rrT")
                trp = ps_tr.tile([P, DC2, NG], BF16, tag="tr")
                for dc in range(DC2):
                    nc.tensor.transpose(trp[:, dc, :],
                                        rr_sb[:, dc * P:(dc + 1) * P],
                                        identB[:NG, :NG])
                nc.vector.tensor_copy(rrT[:], trp[:])
                # gv = sigmoid(rr @ W2_i)
                gv_sb = prep.tile([NG, 2 * D], BF16, tag="gv")
                for half in range(2):
                    gp = ps_v.tile([NG, D], F32, tag="v")
                    for dc in range(DC2):
                        nc.tensor.matmul(
                            gp[:], rrT[:, dc, :],
                            W2_sb[:, dc, half * D:(half + 1) * D],
                            start=(dc == 0), stop=(dc == DC2 - 1))
                    nc.scalar.activation(gv_sb[:, half * D:(half + 1) * D],
                                         gp[:], AF.Sigmoid)
                # gvT then v = gv * a_i  (broadcast over n)
                trp2 = ps_tr.tile([P, DC2, NG], BF16, tag="tr")
                for dc in range(DC2):
                    nc.tensor.transpose(trp2[:, dc, :],
                                        gv_sb[:, dc * P:(dc + 1) * P],
                                        identB[:NG, :NG])
                vT = prep.tile([P, DC2, NG], BF16, tag="vT")
                nc.vector.tensor_tensor(
                    vT[:], trp2[:],
                    aT_sb[:, :, i:i + 1].broadcast_to((P, DC2, NG)), OP.mult)
                # U_i(side) = W_i^T-contracted projectors, both sides at once:
                # lhsT = WtT_i chunk [d,128k], rhs = vT[:, {dc, dc+4}, :]
                for kc in range(KC):
                    up = ps_v.tile([P, 2, NG], F32, tag="v")
                    for dc in range(EC):
                        nc.tensor.matmul(
                            up[:], WT_sb[:, dc, kc * P:(kc + 1) * P],
                            vT[:, dc:dc + EC + 1:EC, :],
                            start=(dc == 0), stop=(dc == EC - 1))
                    nc.vector.tensor_copy(U_all[:, kc, :, i, :], up[:])
            _dma_split(nc, [nc.scalar], Wt2_sb[:],
                       Wt2.rearrange("(c p) d -> p c d", p=P), 2)
            # prefill the ones rows of both ring buffers of the LR stacks
            for _ in range(2):
                Lt = small.tile([66, E], F32R, tag="Lt")
                nc.vector.memset(Lt[0:65:32, :].bitcast(F32), 1.0)
                Rt = small.tile([66, E], F32R, tag="Rt")
                nc.vector.memset(Rt[1:66:32, :].bitcast(F32), 1.0)

        def phase2(n, st):
            """back half: LR rows, scores, softmax, F, out"""
            X_sb = st["X_sb"]; Xt_sb = st["Xt_sb"]
            adj_sb = st["adj_sb"]; m2 = st["m2"]; m3 = st["m3"]

            pLR = ps_lr.tile([2 * NT, E], F32, tag="lr")
            for kc in range(KC):
                nc.tensor.matmul(pLR[:], U_all[:, kc, :, :, n], Xt_sb[:, kc, :],
                                 start=(kc == 0), stop=(kc == KC - 1))
            # stacks: Lt rows {32t: one, 32t+1: L_t}, Rt rows {32t: R_t, 32t+1: one}
            Lt = small.tile([66, E], F32R, tag="Lt")
            Rt = small.tile([66, E], F32R, tag="Rt")
            nc.scalar.copy(Lt[1:66:32, :], pLR[0:NT, :])
            nc.scalar.copy(Rt[0:65:32, :], pLR[NT:2 * NT, :])

            E_sb = sbuf.tile([P, EC, E], BF16, tag="E")
            rs = small.tile([P, EC], F32, tag="rs")
            rsr = small.tile([P, EC], F32, tag="rsr")
            for ic in range(EC):
                pv = []
                for t in range(NT):
                    pvt = ps_v.tile([P, E], F32, tag="v")
                    nc.tensor.matmul(pvt[:], Lt[32 * t:32 * t + 2, ic * P:(ic + 1) * P],
                                     Rt[32 * t:32 * t + 2, :], start=True, stop=True)
                    pv.append(pvt)
                nc.vector.copy_predicated(pv[0][:], m2[:, ic, :], pv[1][:])
                nc.vector.copy_predicated(pv[0][:], m3[:, ic, :], pv[2][:])
                # lrelu in place: max(0.2*x, x) (no ACT table flip)
                nc.gpsimd.scalar_tensor_tensor(pv[0][:], pv[0][:], LRELU_SLOPE,
                                               pv[0][:], OP.mult, OP.max)
                # adj==0 cells -> NEG_BIG: copy typed cells over a -BIG fill,
                # predicated directly on the raw int8 adj (nonzero = typed)
                negt = small.tile([P, E], F32, tag="sc")
                nc.gpsimd.memset(negt[:], NEG_BIG)
                nc.vector.copy_predicated(negt[:], adj_sb[:, ic, :], pv[0][:])
                nc.scalar.activation(E_sb[:, ic, :], negt[:], AF.Exp,
                                     accum_out=rs[:, ic:ic + 1])
                nc.vector.reciprocal(rsr[:, ic:ic + 1], rs[:, ic:ic + 1])
                nc.vector.tensor_scalar(E_sb[:, ic, :], E_sb[:, ic, :],
                                        rsr[:, ic:ic + 1], None, OP.mult)

            F_sb = sbuf.tile([P, KC, E], BF16, tag="F")
            for kc in range(KC):
                pF = ps_big.tile([P, E], F32, tag="big")
                for ec in range(EC):
                    nc.tensor.matmul(pF[:], X_sb[:, ec, kc * P:(kc + 1) * P],
                                     E_sb[:, ec, :],
                                     start=(ec == 0), stop=(ec == EC - 1))
                nc.scalar.copy(F_sb[:, kc, :], pF[:])

            for jc in range(EC):
                pO = ps_big.tile([P, D], F32, tag="big")
                for kc in range(KC):
                    nc.tensor.matmul(pO[:], F_sb[:, kc, jc * P:(jc + 1) * P],
                                     Wt2_sb[:, kc, :],
                                     start=(kc == 0), stop=(kc == KC - 1))
                o_sb = small.tile([P, D], BF16, tag="osb")
                if jc % 2 == 0:
                    nc.gpsimd.tensor_copy(o_sb[:], pO[:])
                else:
                    nc.scalar.copy(o_sb[:], pO[:])
                nc.sync.dma_start(out[n, jc * P:(jc + 1) * P, :], o_sb[:])

        def body_all(_iv=None):
            sts = {}
            sts[0] = phase1(0)
            sts[1] = phase1(1)
            run_prep()
            for n in range(NG):
                if n + 2 < NG:
                    sts[n + 2] = phase1(n + 2)
                phase2(n, sts.pop(n))

        if reps == 1:
            body_all()
        else:
            with tc.For_i(0, reps, 1) as _iv:
                body_all(_iv)
    return nc


_NC_CACHE = {}
TRACE = False
_LAST = {}


def _get_nc():
    if "nc" not in _NC_CACHE:
        nc = bacc.Bacc("TRN2", target_bir_lowering=False, debug=False)
        build(nc)
        nc.compile()
        _NC_CACHE["nc"] = nc
    return _NC_CACHE["nc"]


def kernel(input_state, adj, entity_mask, query_vec, W_type, a_type,
           qattn_W1, qattn_W2):
    import ml_dtypes
    from concourse import bass_utils
    bf16 = ml_dtypes.bfloat16
    nc = _get_nc()

    x_bf = np.ascontiguousarray(input_state, dtype=np.float32).astype(bf16)
    adj_i8 = np.ascontiguousarray(adj).astype(np.int8)
    qv = np.ascontiguousarray(query_vec, dtype=np.float32).astype(bf16)
    # aT[p, dc2, t] = a_type[t, dc2*128 + p]
    aT = np.ascontiguousarray(
        np.transpose(np.asarray(a_type, np.float32).reshape(NT, DC2, P),
                     (2, 1, 0)))
    WtT = np.ascontiguousarray(
        np.transpose(np.asarray(W_type, np.float32), (0, 2, 1))).astype(bf16)
    Wt2 = np.ascontiguousarray(np.asarray(W_type, np.float32)[2]).astype(bf16)
    W1_bf = np.ascontiguousarray(qattn_W1, dtype=np.float32).astype(bf16)
    W2_bf = np.ascontiguousarray(qattn_W2, dtype=np.float32).astype(bf16)

    in_maps = []
    for c in range(N_CORES):
        sl = slice(c * NG, (c + 1) * NG)
        # qT[p, kc, n] = qv[n, kc*128 + p]
        qT = np.ascontiguousarray(
            np.transpose(qv[sl].reshape(NG, KC, P), (2, 1, 0)))
        in_maps.append({
            "x": x_bf[sl], "adj": adj_i8[sl], "qT": qT, "aT": aT,
            "WtT": WtT, "Wt2": Wt2, "W1": W1_bf, "W2q": W2_bf,
        })
    res = bass_utils.run_bass_kernel_spmd(nc, in_maps, core_ids=list(range(N_CORES)),
                                          trace=TRACE, stitch_traces=TRACE)
    _LAST["exec_ns"] = res.exec_time_ns
    _LAST["mean_ns"] = res.mean_exec_time_ns
    _LAST["trace"] = res.instructions_and_trace
    _LAST["scope_times"] = res.per_core_scope_times
    out = np.concatenate([np.asarray(r["out"], np.float32) for r in res.results],
                         axis=0)
    return out


# revision 15
# speedup vs baseline: 1.4823x; 1.0443x over previous
"""GAT self-attention Trainium2 kernel (v2: bf16 datapath, overlapped prep).

Full inputs -> shard graphs over 8 NeuronCores -> full output.

Math (per graph n, reference reformulated):
  g_i = sigmoid(relu(q @ W1_i) @ W2_i)            [2d]
  u_i^L = W_i @ (g_i[:d] * a_i[:d])               [k]   (left projector)
  u_i^R = W_i @ (g_i[d:] * a_i[d:])               [k]   (right projector)
  left_i = X @ u_i^L ; right_i = X @ u_i^R        [E]
  score[i,j] = lrelu(left_t[i] + right_t[j]), t = adj[i,j]; -BIG if adj==0
  E = exp(score); rs = rowsum(E); En = E / rs[:,None]
  out = (En^T @ X) @ W_2

Host marshaling: inputs/weights cast to bf16 (adj to int8, lossless); weight
matrices pre-transposed so the device never transposes weights; query vectors
pre-packed in transposed layout. Device does all matmuls/softmax; output is
written fp32 straight from PSUM.
"""
import numpy as np
from contextlib import ExitStack

import concourse.bass as bass
import concourse.tile as tile
from concourse import mybir, bacc
from concourse.masks import make_identity

F32 = mybir.dt.float32
F32R = mybir.dt.float32r
BF16 = mybir.dt.bfloat16
U8 = mybir.dt.uint8
I8 = mybir.dt.int8
AF = mybir.ActivationFunctionType
OP = mybir.AluOpType

N_CORES = 8
N, E, K, D = 64, 512, 512, 512   # graphs, entities, in_dim, out_dim
NG = N // N_CORES                # graphs per core
NT = 3                           # edge types
P = 128
EC = E // P                      # 4 partition chunks of E
KC = K // P
DC2 = (2 * D) // P               # 8 chunks of the 2d gate dim
NEG_BIG = -200.0
LRELU_SLOPE = 0.2
USE_ACT_LRELU = True             # leaky-relu on ACT engine (alpha operand)


def _dma_split(nc, engs, dst, src, pieces):
    """Split a load along dim 1 of dst across the given engine queues."""
    n0 = dst.shape[1]
    step = max(1, n0 // pieces)
    i = 0
    c = 0
    while i < n0:
        j = min(n0, i + step)
        engs[c % len(engs)].dma_start(dst[:, i:j], src[:, i:j])
        i = j
        c += 1


def build(nc, reps=1):
    x = nc.dram_tensor("x", [NG, E, K], BF16, kind="ExternalInput").ap()
    adj = nc.dram_tensor("adj", [NG, E, E], I8, kind="ExternalInput").ap()
    qT = nc.dram_tensor("qT", [P, KC, NG], BF16, kind="ExternalInput").ap()
    aT = nc.dram_tensor("aT", [P, DC2, NT], F32, kind="ExternalInput").ap()
    WtT = nc.dram_tensor("WtT", [NT, D, K], BF16, kind="ExternalInput").ap()
    Wt2 = nc.dram_tensor("Wt2", [K, D], BF16, kind="ExternalInput").ap()
    W1 = nc.dram_tensor("W1", [NT, K, 2 * D], BF16, kind="ExternalInput").ap()
    W2q = nc.dram_tensor("W2q", [NT, 2 * D, 2 * D], BF16, kind="ExternalInput").ap()
    out = nc.dram_tensor("out", [NG, E, D], BF16, kind="ExternalOutput").ap()
    nc._gat_io = (x, adj, qT, aT, WtT, Wt2, W1, W2q, out)
    _build_once(nc, reps)


def _build_once(nc, reps=1):
    x, adj, qT_d, aT_d, WtT, Wt2, W1, W2q, out = nc._gat_io
    with tile.TileContext(nc) as tc, ExitStack() as ctx:
        # ---------------- pools ----------------
        pers = ctx.enter_context(tc.tile_pool(name="pers", bufs=1))
        prep = ctx.enter_context(tc.tile_pool(name="prep", bufs=2))
        deep = ctx.enter_context(tc.tile_pool(name="deep", bufs=3))
        sbuf = ctx.enter_context(tc.tile_pool(name="sbuf", bufs=2))
        small = ctx.enter_context(tc.tile_pool(name="small", bufs=2))
        ps_lr = ctx.enter_context(tc.tile_pool(name="ps_lr", bufs=2, space="PSUM"))
        ps_v = ctx.enter_context(tc.tile_pool(name="ps_v", bufs=4, space="PSUM"))
        ps_big = ctx.enter_context(tc.tile_pool(name="ps_big", bufs=2, space="PSUM"))

        # ---------------- persistent tiles ----------------
        identB = pers.tile([P, P], BF16)
        make_identity(nc, identB[:])
        neg_col = pers.tile([P, 1], F32)
        nc.vector.memset(neg_col[:], NEG_BIG)
        # U_all[k%128, kc, s, i, n]: projectors, order (L1,L2,L3,R1,R2,R3)
        U_all = pers.tile([P, KC, 2, NT, NG], F32R)
        qT_sb = pers.tile([P, KC, NG], BF16)
        aT_sb = pers.tile([P, DC2, NT], F32)
        Wt2_sb = pers.tile([P, KC, D], BF16)

        def phase1(n):
            """weight-independent front half: loads, Xt, masks"""
            X_sb = deep.tile([P, EC, K], BF16, tag="X")
            nc.sync.dma_start(X_sb[:, 0:2], x[n].rearrange("(c p) k -> p c k", p=P)[:, 0:2])
            nc.sync.dma_start(X_sb[:, 2:4], x[n].rearrange("(c p) k -> p c k", p=P)[:, 2:4])
            adj_sb = deep.tile([P, EC, E], I8, tag="adj")
            nc.sync.dma_start(adj_sb[:], adj[n].rearrange("(c p) j -> p c j", p=P))

            Xt_sb = deep.tile([P, KC, E], BF16, tag="Xt")
            nc.sync.dma_start_transpose(Xt_sb[:], x[n])

            m2 = deep.tile([P, EC, E], U8, tag="m2")
            m3 = deep.tile([P, EC, E], U8, tag="m3")
            nc.gpsimd.tensor_scalar(m2[:], adj_sb[:], 2, None, OP.is_equal)
            nc.gpsimd.tensor_scalar(m3[:], adj_sb[:], 3, None, OP.is_equal)
            return dict(X_sb=X_sb, Xt_sb=Xt_sb, adj_sb=adj_sb, m2=m2, m3=m3)

        def run_prep():
            nc.scalar.dma_start(qT_sb[:], qT_d)
            nc.scalar.dma_start(aT_sb[:], aT_d)
            for i in range(NT):
                # weight loads first so DMA stays saturated
                W1_sb = prep.tile([P, KC, 2 * D], BF16, tag="w1")
                _dma_split(nc, [nc.scalar], W1_sb[:],
                           W1[i].rearrange("(c p) f -> p c f", p=P), 2)
                W2_sb = prep.tile([P, DC2, 2 * D], BF16, tag="w2")
                _dma_split(nc, [nc.scalar], W2_sb[:],
                           W2q[i].rearrange("(c p) f -> p c f", p=P), 4)
                WT_sb = prep.tile([P, EC, K], BF16, tag="wt")
                _dma_split(nc, [nc.scalar], WT_sb[:],
                           WtT[i].rearrange("(c p) k -> p c k", p=P), 2)

                # rr = relu(q @ W1_i):  [NG, 2d] in two 512-halves
                rr_sb = prep.tile([NG, 2 * D], BF16, tag="rr")
                for half in range(2):
                    rp = ps_v.tile([NG, D], F32, tag="v")
                    for kc in range(KC):
                        nc.tensor.matmul(
                            rp[:], qT_sb[:, kc, :],
                            W1_sb[:, kc, half * D:(half + 1) * D],
                            start=(kc == 0), stop=(kc == KC - 1))
                    nc.scalar.activation(rr_sb[:, half * D:(half + 1) * D],
                                         rp[:], AF.Relu)
                # rrT[(2d)%128, dc, n] via PE transposes
                rrT = prep.tile([P, DC2, NG], BF16, tag="rrT")
                trp = ps_big.tile([P, DC2, NG], BF16, tag="big")
                for dc in range(DC2):
                    nc.tensor.transpose(trp[:, dc, :],
                                        rr_sb[:, dc * P:(dc + 1) * P],
                                        identB[:NG, :NG])
                nc.vector.tensor_copy(rrT[:], trp[:])
                # gv = sigmoid(rr @ W2_i)
                gv_sb = prep.tile([NG, 2 * D], BF16, tag="gv")
                for half in range(2):
                    gp = ps_v.tile([NG, D], F32, tag="v")
                    for dc in range(DC2):
                        nc.tensor.matmul(
                            gp[:], rrT[:, dc, :],
                            W2_sb[:, dc, half * D:(half + 1) * D],
                            start=(dc == 0), stop=(dc == DC2 - 1))
                    nc.scalar.activation(gv_sb[:, half * D:(half + 1) * D],
                                         gp[:], AF.Sigmoid)
                # gvT then v = gv * a_i  (broadcast over n)
                trp2 = ps_big.tile([P, DC2, NG], BF16, tag="big")
                for dc in range(DC2):
                    nc.tensor.transpose(trp2[:, dc, :],
                                        gv_sb[:, dc * P:(dc + 1) * P],
                                        identB[:NG, :NG])
                vT = prep.tile([P, DC2, NG], BF16, tag="vT")
                nc.vector.tensor_tensor(
                    vT[:], trp2[:],
                    aT_sb[:, :, i:i + 1].broadcast_to((P, DC2, NG)), OP.mult)
                # U_i(side) = W_i^T-contracted projectors, both sides at once:
                # lhsT = WtT_i chunk [d,128k], rhs = vT[:, {dc, dc+4}, :]
                for kc in range(KC):
                    up = ps_v.tile([P, 2, NG], F32, tag="v")
                    for dc in range(EC):
                        nc.tensor.matmul(
                            up[:], WT_sb[:, dc, kc * P:(kc + 1) * P],
                            vT[:, dc:dc + EC + 1:EC, :],
                            start=(dc == 0), stop=(dc == EC - 1))
                    nc.vector.tensor_copy(U_all[:, kc, :, i, :], up[:])
            _dma_split(nc, [nc.scalar], Wt2_sb[:],
                       Wt2.rearrange("(c p) d -> p c d", p=P), 2)
            # prefill the ones rows of both ring buffers of the LR stacks
            for _ in range(2):
                Lt = small.tile([66, E], F32R, tag="Lt")
                nc.vector.memset(Lt[0:65:32, :].bitcast(F32), 1.0)
                Rt = small.tile([66, E], F32R, tag="Rt")
                nc.vector.memset(Rt[1:66:32, :].bitcast(F32), 1.0)

        def phase2(n, st):
            """back half: LR rows, scores, softmax, F, out"""
            X_sb = st["X_sb"]; Xt_sb = st["Xt_sb"]
            adj_sb = st["adj_sb"]; m2 = st["m2"]; m3 = st["m3"]

            pLR = ps_lr.tile([2 * NT, E], F32, tag="lr")
            for kc in range(KC):
                nc.tensor.matmul(pLR[:], U_all[:, kc, :, :, n], Xt_sb[:, kc, :],
                                 start=(kc == 0), stop=(kc == KC - 1))
            # stacks: Lt rows {32t: one, 32t+1: L_t}, Rt rows {32t: R_t, 32t+1: one}
            Lt = small.tile([66, E], F32R, tag="Lt")
            Rt = small.tile([66, E], F32R, tag="Rt")
            nc.scalar.copy(Lt[1:66:32, :], pLR[0:NT, :])
            nc.scalar.copy(Rt[0:65:32, :], pLR[NT:2 * NT, :])

            E_sb = sbuf.tile([P, EC, E], BF16, tag="E")
            rs = small.tile([P, EC], F32, tag="rs")
            rsr = small.tile([P, EC], F32, tag="rsr")
            negt = sbuf.tile([P, EC, E], F32, tag="negt")
            nc.gpsimd.memset(negt[:], NEG_BIG)
            for ic in range(EC):
                pv = []
                for t in range(NT):
                    pvt = ps_v.tile([P, E], F32, tag="v")
                    nc.tensor.matmul(pvt[:], Lt[32 * t:32 * t + 2, ic * P:(ic + 1) * P],
                                     Rt[32 * t:32 * t + 2, :], start=True, stop=True)
                    pv.append(pvt)
                nc.vector.copy_predicated(pv[0][:], m2[:, ic, :], pv[1][:])
                nc.vector.copy_predicated(pv[0][:], m3[:, ic, :], pv[2][:])
                # lrelu in place: max(0.2*x, x) (no ACT table flip)
                lr_eng = nc.gpsimd if ic < 2 else nc.vector
                lr_eng.scalar_tensor_tensor(pv[0][:], pv[0][:], LRELU_SLOPE,
                                            pv[0][:], OP.mult, OP.max)
                # adj==0 cells -> NEG_BIG: copy typed cells over a -BIG fill,
                # predicated directly on the raw int8 adj (nonzero = typed)
                nc.vector.copy_predicated(negt[:, ic, :], adj_sb[:, ic, :],
                                          pv[0][:])
                nc.scalar.activation(E_sb[:, ic, :], negt[:, ic, :], AF.Exp,
                                     accum_out=rs[:, ic:ic + 1])
                nc.vector.reciprocal(rsr[:, ic:ic + 1], rs[:, ic:ic + 1])
                nc.vector.tensor_scalar(E_sb[:, ic, :], E_sb[:, ic, :],
                                        rsr[:, ic:ic + 1], None, OP.mult)

            F_sb = sbuf.tile([P, KC, E], BF16, tag="F")
            for kc in range(KC):
                pF = ps_big.tile([P, E], F32, tag="big")
                for ec in range(EC):
                    nc.tensor.matmul(pF[:], X_sb[:, ec, kc * P:(kc + 1) * P],
                                     E_sb[:, ec, :],
                                     start=(ec == 0), stop=(ec == EC - 1))
                nc.scalar.copy(F_sb[:, kc, :], pF[:])

            for jc in range(EC):
                pO = ps_big.tile([P, D], F32, tag="big")
                for kc in range(KC):
                    nc.tensor.matmul(pO[:], F_sb[:, kc, jc * P:(jc + 1) * P],
                                     Wt2_sb[:, kc, :],
                                     start=(kc == 0), stop=(kc == KC - 1))
                o_sb = small.tile([P, D], BF16, tag="osb")
                nc.scalar.copy(o_sb[:], pO[:])
                nc.sync.dma_start(out[n, jc * P:(jc + 1) * P, :], o_sb[:])

        def body_all(_iv=None):
            sts = {}
            sts[0] = phase1(0)
            sts[1] = phase1(1)
            run_prep()
            for n in range(NG):
                phase2(n, sts.pop(n))
                if n + 2 < NG:
                    sts[n + 2] = phase1(n + 2)

        if reps == 1:
            body_all()
        else:
            with tc.For_i(0, reps, 1) as _iv:
                body_all(_iv)
    return nc


_NC_CACHE = {}
TRACE = False
_LAST = {}


def _get_nc():
    if "nc" not in _NC_CACHE:
        nc = bacc.Bacc("TRN2", target_bir_lowering=False, debug=False)
        build(nc)
        nc.compile()
        _NC_CACHE["nc"] = nc
    return _NC_CACHE["nc"]


def kernel(input_state, adj, entity_mask, query_vec, W_type, a_type,
           qattn_W1, qattn_W2):
    import ml_dtypes
    from concourse import bass_utils
    bf16 = ml_dtypes.bfloat16
    nc = _get_nc()

    x_bf = np.ascontiguousarray(input_state, dtype=np.float32).astype(bf16)
    adj_i8 = np.ascontiguousarray(adj).astype(np.int8)
    qv = np.ascontiguousarray(query_vec, dtype=np.float32).astype(bf16)
    # aT[p, dc2, t] = a_type[t, dc2*128 + p]
    aT = np.ascontiguousarray(
        np.transpose(np.asarray(a_type, np.float32).reshape(NT, DC2, P),
                     (2, 1, 0)))
    WtT = np.ascontiguousarray(
        np.transpose(np.asarray(W_type, np.float32), (0, 2, 1))).astype(bf16)
    Wt2 = np.ascontiguousarray(np.asarray(W_type, np.float32)[2]).astype(bf16)
    W1_bf = np.ascontiguousarray(qattn_W1, dtype=np.float32).astype(bf16)
    W2_bf = np.ascontiguousarray(qattn_W2, dtype=np.float32).astype(bf16)

    in_maps = []
    for c in range(N_CORES):
        sl = slice(c * NG, (c + 1) * NG)
        # qT[p, kc, n] = qv[n, kc*128 + p]
        qT = np.ascontiguousarray(
            np.transpose(qv[sl].reshape(NG, KC, P), (2, 1, 0)))
        in_maps.append({
            "x": x_bf[sl], "adj": adj_i8[sl], "qT": qT, "aT": aT,
            "WtT": WtT, "Wt2": Wt2, "W1": W1_bf, "W2q": W2_bf,
        })
    res = bass_utils.run_bass_kernel_spmd(nc, in_maps, core_ids=list(range(N_CORES)),
                                          trace=TRACE, stitch_traces=TRACE)
    _LAST["exec_ns"] = res.exec_time_ns
    _LAST["mean_ns"] = res.mean_exec_time_ns
    _LAST["trace"] = res.instructions_and_trace
    _LAST["scope_times"] = res.per_core_scope_times
    out = np.concatenate([np.asarray(r["out"], np.float32) for r in res.results],
                         axis=0)
    return out


# revision 20
# speedup vs baseline: 1.5582x; 1.0512x over previous
"""GAT self-attention Trainium2 kernel (v2: bf16 datapath, overlapped prep).

Full inputs -> shard graphs over 8 NeuronCores -> full output.

Math (per graph n, reference reformulated):
  g_i = sigmoid(relu(q @ W1_i) @ W2_i)            [2d]
  u_i^L = W_i @ (g_i[:d] * a_i[:d])               [k]   (left projector)
  u_i^R = W_i @ (g_i[d:] * a_i[d:])               [k]   (right projector)
  left_i = X @ u_i^L ; right_i = X @ u_i^R        [E]
  score[i,j] = lrelu(left_t[i] + right_t[j]), t = adj[i,j]; -BIG if adj==0
  E = exp(score); rs = rowsum(E); En = E / rs[:,None]
  out = (En^T @ X) @ W_2

Host marshaling: inputs/weights cast to bf16 (adj to int8, lossless); weight
matrices pre-transposed so the device never transposes weights; query vectors
pre-packed in transposed layout. Device does all matmuls/softmax; output is
written fp32 straight from PSUM.
"""
import numpy as np
from contextlib import ExitStack

import concourse.bass as bass
import concourse.tile as tile
from concourse import mybir, bacc
from concourse.masks import make_identity

F32 = mybir.dt.float32
F32R = mybir.dt.float32r
BF16 = mybir.dt.bfloat16
U8 = mybir.dt.uint8
I8 = mybir.dt.int8
AF = mybir.ActivationFunctionType
OP = mybir.AluOpType

N_CORES = 8
N, E, K, D = 64, 512, 512, 512   # graphs, entities, in_dim, out_dim
NG = N // N_CORES                # graphs per core
NT = 3                           # edge types
P = 128
EC = E // P                      # 4 partition chunks of E
KC = K // P
DC2 = (2 * D) // P               # 8 chunks of the 2d gate dim
NEG_BIG = -200.0
LRELU_SLOPE = 0.2
USE_ACT_LRELU = True             # leaky-relu on ACT engine (alpha operand)


def _dma_split(nc, engs, dst, src, pieces):
    """Split a load along dim 1 of dst across the given engine queues."""
    n0 = dst.shape[1]
    step = max(1, n0 // pieces)
    i = 0
    c = 0
    while i < n0:
        j = min(n0, i + step)
        engs[c % len(engs)].dma_start(dst[:, i:j], src[:, i:j])
        i = j
        c += 1


def build(nc, reps=1):
    x = nc.dram_tensor("x", [NG, E, K], BF16, kind="ExternalInput").ap()
    adj = nc.dram_tensor("adj", [NG, E, E], I8, kind="ExternalInput").ap()
    qT = nc.dram_tensor("qT", [P, KC, NG], BF16, kind="ExternalInput").ap()
    aT = nc.dram_tensor("aT", [P, DC2, NT], F32, kind="ExternalInput").ap()
    WtT = nc.dram_tensor("WtT", [NT, D, K], BF16, kind="ExternalInput").ap()
    Wt2 = nc.dram_tensor("Wt2", [K, D], BF16, kind="ExternalInput").ap()
    W1 = nc.dram_tensor("W1", [NT, K, 2 * D], BF16, kind="ExternalInput").ap()
    W2q = nc.dram_tensor("W2q", [NT, 2 * D, 2 * D], BF16, kind="ExternalInput").ap()
    out = nc.dram_tensor("out", [NG, E, D], BF16, kind="ExternalOutput").ap()
    nc._gat_io = (x, adj, qT, aT, WtT, Wt2, W1, W2q, out)
    _build_once(nc, reps)


def _build_once(nc, reps=1):
    x, adj, qT_d, aT_d, WtT, Wt2, W1, W2q, out = nc._gat_io
    with tile.TileContext(nc) as tc, ExitStack() as ctx:
        # ---------------- pools ----------------
        pers = ctx.enter_context(tc.tile_pool(name="pers", bufs=1))
        prep = ctx.enter_context(tc.tile_pool(name="prep", bufs=2))
        deep = ctx.enter_context(tc.tile_pool(name="deep", bufs=3))
        sbuf = ctx.enter_context(tc.tile_pool(name="sbuf", bufs=2))
        small = ctx.enter_context(tc.tile_pool(name="small", bufs=2))
        ps_lr = ctx.enter_context(tc.tile_pool(name="ps_lr", bufs=2, space="PSUM"))
        ps_v = ctx.enter_context(tc.tile_pool(name="ps_v", bufs=4, space="PSUM"))
        ps_big = ctx.enter_context(tc.tile_pool(name="ps_big", bufs=2, space="PSUM"))

        # ---------------- persistent tiles ----------------
        identB = pers.tile([P, P], BF16)
        make_identity(nc, identB[:])
        neg_col = pers.tile([P, 1], F32)
        nc.vector.memset(neg_col[:], NEG_BIG)
        # U_all[k%128, kc, s, i, n]: projectors, order (L1,L2,L3,R1,R2,R3)
        U_all = pers.tile([P, KC, 2, NT, NG], F32R)
        qT_sb = pers.tile([P, KC, NG], BF16)
        aT_sb = pers.tile([P, DC2, NT], F32)
        Wt2_sb = pers.tile([P, KC, D], BF16)

        def phase1(n):
            """weight-light front half: loads, Xt, masks, H = X @ W2"""
            adj_sb = deep.tile([P, EC, E], I8, tag="adj")
            nc.sync.dma_start(adj_sb[:], adj[n].rearrange("(c p) j -> p c j", p=P))
            Xt_sb = deep.tile([P, KC, E], BF16, tag="Xt")
            nc.sync.dma_start_transpose(Xt_sb[:], x[n])

            m2 = deep.tile([P, EC, E], U8, tag="m2")
            m3 = deep.tile([P, EC, E], U8, tag="m3")
            nc.gpsimd.tensor_scalar(m2[:], adj_sb[:], 2, None, OP.is_equal)
            nc.gpsimd.tensor_scalar(m3[:], adj_sb[:], 3, None, OP.is_equal)

            # H = X @ W2 (score-independent; keeps PE warm during prep)
            H_sb = deep.tile([P, EC, D], BF16, tag="H")
            for ic in range(EC):
                pH = ps_big.tile([P, D], F32, tag="big")
                for kc in range(KC):
                    nc.tensor.matmul(pH[:], Xt_sb[:, kc, ic * P:(ic + 1) * P],
                                     Wt2_sb[:, kc, :],
                                     start=(kc == 0), stop=(kc == KC - 1))
                nc.scalar.copy(H_sb[:, ic, :], pH[:])
            return dict(H_sb=H_sb, Xt_sb=Xt_sb, adj_sb=adj_sb, m2=m2, m3=m3)

        def run_prep():
            _dma_split(nc, [nc.scalar], Wt2_sb[:],
                       Wt2.rearrange("(c p) d -> p c d", p=P), 2)
            nc.scalar.dma_start(qT_sb[:], qT_d)
            nc.scalar.dma_start(aT_sb[:], aT_d)
            for i in range(NT):
                # weight loads first so DMA stays saturated
                W1_sb = prep.tile([P, KC, 2 * D], BF16, tag="w1")
                _dma_split(nc, [nc.scalar], W1_sb[:],
                           W1[i].rearrange("(c p) f -> p c f", p=P), 2)
                W2_sb = prep.tile([P, DC2, 2 * D], BF16, tag="w2")
                _dma_split(nc, [nc.scalar], W2_sb[:],
                           W2q[i].rearrange("(c p) f -> p c f", p=P), 4)
                WT_sb = prep.tile([P, EC, K], BF16, tag="wt")
                _dma_split(nc, [nc.scalar], WT_sb[:],
                           WtT[i].rearrange("(c p) k -> p c k", p=P), 2)

                # rr = relu(q @ W1_i):  [NG, 2d] in two 512-halves
                rr_sb = prep.tile([NG, 2 * D], BF16, tag="rr")
                for half in range(2):
                    rp = ps_v.tile([NG, D], F32, tag="v")
                    for kc in range(KC):
                        nc.tensor.matmul(
                            rp[:], qT_sb[:, kc, :],
                            W1_sb[:, kc, half * D:(half + 1) * D],
                            start=(kc == 0), stop=(kc == KC - 1))
                    nc.scalar.activation(rr_sb[:, half * D:(half + 1) * D],
                                         rp[:], AF.Relu)
                # rrT[(2d)%128, dc, n] via PE transposes
                rrT = prep.tile([P, DC2, NG], BF16, tag="rrT")
                trp = ps_big.tile([P, DC2, NG], BF16, tag="big")
                for dc in range(DC2):
                    nc.tensor.transpose(trp[:, dc, :],
                                        rr_sb[:, dc * P:(dc + 1) * P],
                                        identB[:NG, :NG])
                nc.vector.tensor_copy(rrT[:], trp[:])
                # gv = sigmoid(rr @ W2_i)
                gv_sb = prep.tile([NG, 2 * D], BF16, tag="gv")
                for half in range(2):
                    gp = ps_v.tile([NG, D], F32, tag="v")
                    for dc in range(DC2):
                        nc.tensor.matmul(
                            gp[:], rrT[:, dc, :],
                            W2_sb[:, dc, half * D:(half + 1) * D],
                            start=(dc == 0), stop=(dc == DC2 - 1))
                    nc.scalar.activation(gv_sb[:, half * D:(half + 1) * D],
                                         gp[:], AF.Sigmoid)
                # gvT then v = gv * a_i  (broadcast over n)
                trp2 = ps_big.tile([P, DC2, NG], BF16, tag="big")
                for dc in range(DC2):
                    nc.tensor.transpose(trp2[:, dc, :],
                                        gv_sb[:, dc * P:(dc + 1) * P],
                                        identB[:NG, :NG])
                vT = prep.tile([P, DC2, NG], BF16, tag="vT")
                nc.vector.tensor_tensor(
                    vT[:], trp2[:],
                    aT_sb[:, :, i:i + 1].broadcast_to((P, DC2, NG)), OP.mult)
                # U_i(side) = W_i^T-contracted projectors, both sides at once:
                # lhsT = WtT_i chunk [d,128k], rhs = vT[:, {dc, dc+4}, :]
                for kc in range(KC):
                    up = ps_v.tile([P, 2, NG], F32, tag="v")
                    for dc in range(EC):
                        nc.tensor.matmul(
                            up[:], WT_sb[:, dc, kc * P:(kc + 1) * P],
                            vT[:, dc:dc + EC + 1:EC, :],
                            start=(dc == 0), stop=(dc == EC - 1))
                    nc.vector.tensor_copy(U_all[:, kc, :, i, :], up[:])
            # prefill the ones rows of both ring buffers of the LR stacks
            for _ in range(2):
                Lt = small.tile([66, E], F32R, tag="Lt")
                nc.vector.memset(Lt[0:65:32, :].bitcast(F32), 1.0)
                Rt = small.tile([66, E], F32R, tag="Rt")
                nc.vector.memset(Rt[1:66:32, :].bitcast(F32), 1.0)

        def phase2(n, st):
            """back half: LR rows, scores, softmax, out = E^T @ H"""
            H_sb = st["H_sb"]; Xt_sb = st["Xt_sb"]
            adj_sb = st["adj_sb"]; m2 = st["m2"]; m3 = st["m3"]

            pLR = ps_lr.tile([2 * NT, E], F32, tag="lr")
            for kc in range(KC):
                nc.tensor.matmul(pLR[:], U_all[:, kc, :, :, n], Xt_sb[:, kc, :],
                                 start=(kc == 0), stop=(kc == KC - 1))
            # stacks: Lt rows {32t: one, 32t+1: L_t}, Rt rows {32t: R_t, 32t+1: one}
            Lt = small.tile([66, E], F32R, tag="Lt")
            Rt = small.tile([66, E], F32R, tag="Rt")
            nc.scalar.copy(Lt[1:66:32, :], pLR[0:NT, :])
            nc.scalar.copy(Rt[0:65:32, :], pLR[NT:2 * NT, :])

            E_sb = sbuf.tile([P, EC, E], BF16, tag="E")
            rs = small.tile([P, EC], F32, tag="rs")
            rsr = small.tile([P, EC], F32, tag="rsr")
            negt = sbuf.tile([P, EC, E], F32, tag="negt")
            nc.gpsimd.memset(negt[:], NEG_BIG)
            for ic in range(EC):
                pv = []
                for t in range(NT):
                    pvt = ps_v.tile([P, E], F32, tag="v")
                    nc.tensor.matmul(pvt[:], Lt[32 * t:32 * t + 2, ic * P:(ic + 1) * P],
                                     Rt[32 * t:32 * t + 2, :], start=True, stop=True)
                    pv.append(pvt)
                nc.vector.copy_predicated(pv[0][:], m2[:, ic, :], pv[1][:])
                nc.vector.copy_predicated(pv[0][:], m3[:, ic, :], pv[2][:])
                # lrelu in place: max(0.2*x, x) (no ACT table flip)
                lr_eng = nc.gpsimd if ic < 2 else nc.vector
                lr_eng.scalar_tensor_tensor(pv[0][:], pv[0][:], LRELU_SLOPE,
                                            pv[0][:], OP.mult, OP.max)
                # adj==0 cells -> NEG_BIG: copy typed cells over a -BIG fill,
                # predicated directly on the raw int8 adj (nonzero = typed)
                nc.vector.copy_predicated(negt[:, ic, :], adj_sb[:, ic, :],
                                          pv[0][:])
                nc.scalar.activation(E_sb[:, ic, :], negt[:, ic, :], AF.Exp,
                                     accum_out=rs[:, ic:ic + 1])
                nc.vector.reciprocal(rsr[:, ic:ic + 1], rs[:, ic:ic + 1])
                nc.vector.tensor_scalar(E_sb[:, ic, :], E_sb[:, ic, :],
                                        rsr[:, ic:ic + 1], None, OP.mult)

            for jc in range(EC):
                pO = ps_big.tile([P, D], F32, tag="big")
                for ic in range(EC):
                    nc.tensor.matmul(pO[:], E_sb[:, ic, jc * P:(jc + 1) * P],
                                     H_sb[:, ic, :],
                                     start=(ic == 0), stop=(ic == EC - 1))
                o_sb = small.tile([P, D], BF16, tag="osb")
                nc.scalar.copy(o_sb[:], pO[:])
                nc.sync.dma_start(out[n, jc * P:(jc + 1) * P, :], o_sb[:])

        def body_all(_iv=None):
            sts = {}
            sts[0] = phase1(0)
            sts[1] = phase1(1)
            run_prep()
            for n in range(NG):
                phase2(n, sts.pop(n))
                if n + 2 < NG:
                    sts[n + 2] = phase1(n + 2)

        if reps == 1:
            body_all()
        else:
            with tc.For_i(0, reps, 1) as _iv:
                body_all(_iv)
    return nc


_NC_CACHE = {}
TRACE = False
_LAST = {}


def _get_nc():
    if "nc" not in _NC_CACHE:
        nc = bacc.Bacc("TRN2", target_bir_lowering=False, debug=False)
        build(nc)
        nc.compile()
        _NC_CACHE["nc"] = nc
    return _NC_CACHE["nc"]


def kernel(input_state, adj, entity_mask, query_vec, W_type, a_type,
           qattn_W1, qattn_W2):
    import ml_dtypes
    from concourse import bass_utils
    bf16 = ml_dtypes.bfloat16
    nc = _get_nc()

    x_bf = np.ascontiguousarray(input_state, dtype=np.float32).astype(bf16)
    adj_i8 = np.ascontiguousarray(adj).astype(np.int8)
    qv = np.ascontiguousarray(query_vec, dtype=np.float32).astype(bf16)
    # aT[p, dc2, t] = a_type[t, dc2*128 + p]
    aT = np.ascontiguousarray(
        np.transpose(np.asarray(a_type, np.float32).reshape(NT, DC2, P),
                     (2, 1, 0)))
    WtT = np.ascontiguousarray(
        np.transpose(np.asarray(W_type, np.float32), (0, 2, 1))).astype(bf16)
    Wt2 = np.ascontiguousarray(np.asarray(W_type, np.float32)[2]).astype(bf16)
    W1_bf = np.ascontiguousarray(qattn_W1, dtype=np.float32).astype(bf16)
    W2_bf = np.ascontiguousarray(qattn_W2, dtype=np.float32).astype(bf16)

    in_maps = []
    for c in range(N_CORES):
        sl = slice(c * NG, (c + 1) * NG)
        # qT[p, kc, n] = qv[n, kc*128 + p]
        qT = np.ascontiguousarray(
            np.transpose(qv[sl].reshape(NG, KC, P), (2, 1, 0)))
        in_maps.append({
            "x": x_bf[sl], "adj": adj_i8[sl], "qT": qT, "aT": aT,
            "WtT": WtT, "Wt2": Wt2, "W1": W1_bf, "W2q": W2_bf,
        })
    res = bass_utils.run_bass_kernel_spmd(nc, in_maps, core_ids=list(range(N_CORES)),
                                          trace=TRACE, stitch_traces=TRACE)
    _LAST["exec_ns"] = res.exec_time_ns
    _LAST["mean_ns"] = res.mean_exec_time_ns
    _LAST["trace"] = res.instructions_and_trace
    _LAST["scope_times"] = res.per_core_scope_times
    out = np.concatenate([np.asarray(r["out"], np.float32) for r in res.results],
                         axis=0)
    return out
